# revision 1
# baseline (speedup 1.0000x reference)
"""Multi-head attention kernel for Trainium2, SPMD over 8 NeuronCores.

Sharding: data-parallel over batch (2 groups of 4 cores) x sequence-parallel
over the key/value length within each group (4 slices of 2048). Each core
computes, for its (batch, k-slice): Q/K/V projections (all heads), masked
softmax numerators/denominators over its k-slice, the attention-weighted
values, and a partial final projection. Denominators are AllReduce'd within
each 4-core group on device (split in two so the first overlaps attention);
the 4 partial projected outputs per batch are summed on the host.

Layout notes: activations/weights are cast to bf16 during the DMA load and
transposed on the TensorE (contraction dims must sit on partitions); scores
are computed transposed ([k, q]) so the exp output is directly consumable as
the stationary operand of the AV matmul; the softmax denominator comes from
a ones-column appended to V; no max-subtraction is needed (scores are O(1)),
and masking is a multiplicative bf16 mask applied after exp (exactly
equivalent to the -1e30 additive mask).
"""

import sys

if "/opt/trn_rl_repo" not in sys.path:
    sys.path.insert(0, "/opt/trn_rl_repo")

from contextlib import ExitStack

import numpy as np

import concourse.bass as bass
import concourse.mybir as mybir
import concourse.tile as tile
from concourse import bacc
from concourse.masks import make_identity

B, QL, KL, D, H = 2, 512, 8192, 1024, 8
HD = D // H  # 128
NCORES = 8
GROUPS = [[0, 1, 2, 3], [4, 5, 6, 7]]
KSH = KL // 4  # 2048 k rows per core
SCALE = 1.0 / float(np.sqrt(HD))

F32 = mybir.dt.float32
BF16 = mybir.dt.bfloat16
U8 = mybir.dt.uint8
P = 128
KC = KSH // P  # 16 k chunks of 128
QB = QL // P  # 4 q blocks


def ensure_ntff_hook():
    """Provide antenv.axon_hooks (missing in this image) so trace=True works.

    Mirrors trn_agent_boot._ntff_profile_via_ctypes against the local
    libaxon_pjrt.so. No-op if the real module exists or the .so is absent.
    """
    try:
        import antenv.axon_hooks  # noqa: F401

        return
    except ImportError:
        pass
    import contextlib
    import ctypes
    import types

    mod = types.ModuleType("antenv.axon_hooks")
    holder = [None]
    mod.set_axon_ntff_profile_hook = lambda h: holder.__setitem__(0, h)
    mod.get_axon_ntff_profile_hook = lambda: holder[0]
    try:
        lib = ctypes.CDLL("/opt/axon/libaxon_pjrt.so")
        if hasattr(lib, "axon_start_nrt_profile"):
            lib.axon_start_nrt_profile.argtypes = [
                ctypes.POINTER(ctypes.c_int64),
                ctypes.c_size_t,
            ]
            lib.axon_start_nrt_profile.restype = ctypes.c_int64
            lib.axon_stop_nrt_profile.argtypes = [ctypes.c_char_p]
            lib.axon_stop_nrt_profile.restype = ctypes.c_int64

            @contextlib.contextmanager
            def _hook(output_dir, device_ids):
                import jax

                jax.devices()
                if device_ids:
                    ids = (ctypes.c_int64 * len(device_ids))(*device_ids)
                    rc = lib.axon_start_nrt_profile(ids, len(device_ids))
                else:
                    rc = lib.axon_start_nrt_profile(None, 0)
                if rc != 0:
                    raise RuntimeError(f"axon_start_nrt_profile rc={rc}")
                try:
                    yield
                finally:
                    n = lib.axon_stop_nrt_profile(str(output_dir).encode())
                    print(f"ntff profile: {n} file(s) -> {output_dir}")

            holder[0] = _hook
    except OSError:
        pass
    sys.modules["antenv.axon_hooks"] = mod
    try:
        import antenv

        antenv.axon_hooks = mod
    except ImportError:
        pass


def build_attention_kernel():
    nc = bacc.Bacc(
        "TRN2", target_bir_lowering=False, debug=False, num_devices=NCORES
    )

    xq = nc.declare_dram_parameter("xq", [QL, D], F32, isOutput=False)
    xk = nc.declare_dram_parameter("xk", [KSH, D], F32, isOutput=False)
    xv = nc.declare_dram_parameter("xv", [KSH, D], F32, isOutput=False)
    msk = nc.declare_dram_parameter("msk", [QL, KSH], U8, isOutput=False)
    wq = nc.declare_dram_parameter("wq", [D, D], F32, isOutput=False)
    wk = nc.declare_dram_parameter("wk", [D, D], F32, isOutput=False)
    wv = nc.declare_dram_parameter("wv", [D, D], F32, isOutput=False)
    wf = nc.declare_dram_parameter("wf", [D, D], F32, isOutput=False)
    out = nc.declare_dram_parameter("out", [QL, D], F32, isOutput=True)

    with tile.TileContext(nc) as tc, ExitStack() as ctx:
        consts = ctx.enter_context(tc.tile_pool(name="consts", bufs=1))
        ident = consts.tile([P, P], BF16)
        make_identity(nc, ident)

        # Persistent operand tiles (single-buffered, live for the kernel).
        persist = ctx.enter_context(tc.tile_pool(name="persist", bufs=1))
        wfT = persist.tile([P, H, D], BF16)  # [din in h-chunk, h, dout]
        kT = persist.tile([P, H, KSH], BF16)  # [hd, head, krow]
        qT = persist.tile([P, H, QL], BF16)  # [hd, head, q]
        v_sb = persist.tile([P, KC, H, HD + 1], BF16)  # [krow, kc, h, hd+1]
        maskT = persist.tile([P, KC, QL], BF16)  # [k, kc, q]
        num_sb = persist.tile([P, H, QB, HD], BF16)  # [q, head, qb, hd]
        den0 = persist.tile([P, 12], F32)  # heads 0-2, [q, h*4+qb]
        den1 = persist.tile([P, 20], F32)  # heads 3-7, [q, (h-3)*4+qb]
        rden0 = persist.tile([P, 12], F32)
        rden1 = persist.tile([P, 20], F32)
        sumT = persist.tile([P, H, QL], BF16)  # [hd, head, q]

        wts = ctx.enter_context(tc.tile_pool(name="wts", bufs=1))
        loads = ctx.enter_context(tc.tile_pool(name="loads", bufs=3))
        xts = ctx.enter_context(tc.tile_pool(name="xts", bufs=2))
        mn_pool = ctx.enter_context(tc.tile_pool(name="mn_pool", bufs=4))
        probs_pool = ctx.enter_context(tc.tile_pool(name="probs", bufs=3))
        small = ctx.enter_context(tc.tile_pool(name="small", bufs=4))
        outp = ctx.enter_context(tc.tile_pool(name="outp", bufs=3))
        dram = ctx.enter_context(tc.tile_pool(name="dram", bufs=1, space="DRAM"))

        # One PSUM pool, 8 banks: mm 2x2 + av 4x1. Everything except the AV
        # accumulators shares the [128, 2, 512] "mm" slots.
        psum = ctx.enter_context(tc.tile_pool(name="psum", bufs=1, space="PSUM"))

        def mm_tile(name, dtype=F32):
            return psum.tile([P, 2, 512], dtype, tag="mm", bufs=2, name=name)

        def transpose_w(w_dram, dst, wname):
            """dst[p, cc, dout] = w[dout, cc*128+p] (i.e. dst = W^T), bf16."""
            wns = []
            for rg in range(2):
                wn = loads.tile([P, 4, D], BF16, tag="ld", name=f"wn_{wname}{rg}")
                nc.gpsimd.dma_start(
                    out=wn,
                    in_=w_dram[rg * 512 : (rg + 1) * 512, :].rearrange(
                        "(a p) d -> p a d", p=P
                    ),
                )
                wns.append(wn)
            for cc in range(H):
                pst = mm_tile(f"wt_{wname}_{cc}", BF16)
                for rg in range(2):
                    for j in range(4):
                        nc.tensor.transpose(
                            pst[:, rg, j * P : (j + 1) * P],
                            wns[rg][:, j, cc * P : (cc + 1) * P],
                            ident,
                        )
                nc.vector.tensor_copy(
                    dst[:, cc, :], pst[:].rearrange("p a b -> p (a b)")
                )

        # --- Q path: xq load first (smallest), then Wq ---
        xqn = loads.tile([P, 4, D], BF16, tag="ld")
        nc.gpsimd.dma_start(out=xqn, in_=xq.rearrange("(a p) d -> p a d", p=P))
        wqT = wts.tile([P, H, D], BF16, tag="wT", name="wqT")
        transpose_w(wq, wqT, "q")

        xqT = xts.tile([P, H, QL], BF16, tag="xT")
        for cc2 in range(H // 2):
            pst = mm_tile(f"xqt_{cc2}", BF16)
            for half in range(2):
                cc = cc2 * 2 + half
                for j in range(4):
                    nc.tensor.transpose(
                        pst[:, half, j * P : (j + 1) * P],
                        xqn[:, j, cc * P : (cc + 1) * P],
                        ident,
                    )
            nc.vector.tensor_copy(xqT[:, cc2 * 2 : cc2 * 2 + 2, :], pst[:])
        for m2 in range(H // 2):
            pq = mm_tile(f"pq_{m2}")
            for half in range(2):
                m = m2 * 2 + half
                for cc in range(H):
                    nc.tensor.matmul(
                        pq[:, half, :],
                        wqT[:, cc, m * P : (m + 1) * P],
                        xqT[:, cc, :],
                        start=(cc == 0),
                        stop=(cc == H - 1),
                    )
            nc.any.tensor_copy(out=qT[:, m2 * 2 : m2 * 2 + 2, :], in_=pq[:])

        # --- mask: load+cast per q-block, transpose to [k, q] on PE ---
        mn_tiles = []
        for qb in range(QB):
            mn = mn_pool.tile([P, KSH], BF16, tag="mn", name=f"mn_{qb}")
            nc.gpsimd.dma_start(out=mn, in_=msk[qb * P : (qb + 1) * P, :])
            mn_tiles.append(mn)
        for kc2 in range(KC // 2):
            pst = mm_tile(f"mt_{kc2}", BF16)
            for half in range(2):
                kc = kc2 * 2 + half
                for qb in range(QB):
                    nc.tensor.transpose(
                        pst[:, half, qb * P : (qb + 1) * P],
                        mn_tiles[qb][:, kc * P : (kc + 1) * P],
                        ident,
                    )
            nc.any.tensor_copy(out=maskT[:, kc2 * 2 : kc2 * 2 + 2, :], in_=pst[:])

        # --- Wk, then the K path (stream xk in 512-row chunks) ---
        wkT = wts.tile([P, H, D], BF16, tag="wT", name="wkT")
        transpose_w(wk, wkT, "k")

        for c4 in range(KSH // 512):
            xkn = loads.tile([P, 4, D], BF16, tag="ld", name=f"xkn_{c4}")
            nc.gpsimd.dma_start(
                out=xkn,
                in_=xk[c4 * 512 : (c4 + 1) * 512, :].rearrange("(a p) d -> p a d", p=P),
            )
            xkT = xts.tile([P, H, 512], BF16, tag="xT", name=f"xkT_{c4}")
            for cc2 in range(H // 2):
                pst = mm_tile(f"xkt_{c4}_{cc2}", BF16)
                for half in range(2):
                    cc = cc2 * 2 + half
                    for j in range(4):
                        nc.tensor.transpose(
                            pst[:, half, j * P : (j + 1) * P],
                            xkn[:, j, cc * P : (cc + 1) * P],
                            ident,
                        )
                nc.vector.tensor_copy(xkT[:, cc2 * 2 : cc2 * 2 + 2, :], pst[:])
            for m2 in range(H // 2):
                pk = mm_tile(f"pk_{c4}_{m2}")
                for half in range(2):
                    m = m2 * 2 + half
                    for cc in range(H):
                        nc.tensor.matmul(
                            pk[:, half, :],
                            wkT[:, cc, m * P : (m + 1) * P],
                            xkT[:, cc, :],
                            start=(cc == 0),
                            stop=(cc == H - 1),
                        )
                nc.any.tensor_copy(
                    out=kT[:, m2 * 2 : m2 * 2 + 2, c4 * 512 : (c4 + 1) * 512],
                    in_=pk[:],
                )

        # --- Wv, then the V path ---
        wvT = wts.tile([P, H, D], BF16, tag="wT", name="wvT")
        transpose_w(wv, wvT, "v")

        for c4 in range(KSH // 512):
            xvn = loads.tile([P, 4, D], BF16, tag="ld", name=f"xvn_{c4}")
            nc.gpsimd.dma_start(
                out=xvn,
                in_=xv[c4 * 512 : (c4 + 1) * 512, :].rearrange("(a p) d -> p a d", p=P),
            )
            xvT = xts.tile([P, H, 512], BF16, tag="xT", name=f"xvT_{c4}")
            for cc2 in range(H // 2):
                pst = mm_tile(f"xvt_{c4}_{cc2}", BF16)
                for half in range(2):
                    cc = cc2 * 2 + half
                    for j in range(4):
                        nc.tensor.transpose(
                            pst[:, half, j * P : (j + 1) * P],
                            xvn[:, j, cc * P : (cc + 1) * P],
                            ident,
                        )
                nc.vector.tensor_copy(xvT[:, cc2 * 2 : cc2 * 2 + 2, :], pst[:])
            for mkl in range(4):
                mk = c4 * 4 + mkl
                pv = mm_tile(f"pv_{mk}")
                for n in range(2):
                    for cc in range(H):
                        nc.tensor.matmul(
                            pv[:, n, :],
                            xvT[:, cc, mkl * P : (mkl + 1) * P],
                            wvT[:, cc, n * 512 : (n + 1) * 512],
                            start=(cc == 0),
                            stop=(cc == H - 1),
                        )
                nc.any.tensor_copy(
                    out=v_sb[:, mk, :, 0:HD],
                    in_=pv[:].rearrange("p a (b c) -> p (a b) c", b=4),
                )
        nc.vector.memset(v_sb[:, :, :, HD], 1.0)

        transpose_w(wf, wfT, "f")

        # --- attention per head; exp batched over 2 k-chunks ---
        def attention_head(h, den_tile):
            avs = [
                psum.tile([P, HD + 1], F32, tag="av", bufs=4, name=f"av_{h}_{qb}")
                for qb in range(QB)
            ]
            for kc2 in range(KC // 2):
                ps = mm_tile(f"ps_{h}_{kc2}")
                for half in range(2):
                    kc = kc2 * 2 + half
                    nc.tensor.matmul(
                        ps[:, half, :],
                        kT[:, h, kc * P : (kc + 1) * P],
                        qT[:, h, :],
                        start=True,
                        stop=True,
                    )
                probs = probs_pool.tile(
                    [P, 2, 512], BF16, tag="probs", name=f"pr_{h}_{kc2}"
                )
                nc.scalar.activation(
                    probs[:], ps[:], mybir.ActivationFunctionType.Exp, scale=SCALE
                )
                nc.vector.tensor_mul(
                    probs[:], probs[:], maskT[:, kc2 * 2 : kc2 * 2 + 2, :]
                )
                for half in range(2):
                    kc = kc2 * 2 + half
                    for qb in range(QB):
                        nc.tensor.matmul(
                            avs[qb][:],
                            probs[:, half, qb * P : (qb + 1) * P],
                            v_sb[:, kc, h, :],
                            start=(kc == 0),
                            stop=(kc == KC - 1),
                        )
            hh = h if h < 3 else h - 3
            for qb in range(QB):
                nc.any.tensor_copy(out=num_sb[:, h, qb, :], in_=avs[qb][:, 0:HD])
                nc.any.tensor_copy(
                    out=den_tile[:, hh * 4 + qb : hh * 4 + qb + 1],
                    in_=avs[qb][:, HD : HD + 1],
                )

        def den_allreduce(den_tile, rden_tile, idx):
            ncols = den_tile.shape[-1]
            den_in = dram.tile([P, ncols], F32, name=f"den_in{idx}")
            den_out = dram.tile([P, ncols], F32, name=f"den_out{idx}")
            nc.sync.dma_start(out=den_in[:], in_=den_tile[:])
            nc.gpsimd.collective_compute(
                "AllReduce",
                mybir.AluOpType.add,
                replica_groups=GROUPS,
                ins=[den_in.opt()],
                outs=[den_out.opt()],
            )
            nc.sync.dma_start(out=rden_tile[:], in_=den_out[:])
            # guard fully-masked rows (reference wipes them to 0): 0/eps -> 0
            nc.vector.tensor_scalar_max(rden_tile[:], rden_tile[:], 1e-30)
            nc.vector.reciprocal(rden_tile[:], rden_tile[:])

        def norm_head(h, rden_tile):
            hh = h if h < 3 else h - 3
            snorms = []
            for qb in range(QB):
                snorm = small.tile([P, HD], BF16, tag="snorm", name=f"sn_{h}_{qb}")
                nc.vector.tensor_scalar_mul(
                    snorm[:],
                    num_sb[:, h, qb, :],
                    rden_tile[:, hh * 4 + qb : hh * 4 + qb + 1],
                )
                snorms.append(snorm)
            pst = mm_tile(f"st_{h}", BF16)
            for qb in range(QB):
                nc.tensor.transpose(
                    pst[:, 0, qb * P : (qb + 1) * P], snorms[qb][:], ident
                )
            nc.any.tensor_copy(out=sumT[:, h, :], in_=pst[:, 0, :])

        for h in range(3):
            attention_head(h, den0)
        den_allreduce(den0, rden0, 0)
        for h in range(3, H):
            attention_head(h, den1)
        for h in range(3):
            norm_head(h, rden0)
        den_allreduce(den1, rden1, 1)
        for h in range(3, H):
            norm_head(h, rden1)

        # First 4 output tiles accumulate heads 0-2 on the freed AV psum
        # slots right after norm of those heads, overlapping the second
        # collective; heads 3-7 complete the groups afterwards.
        po_av = {}
        for qb in range(2):
            for n in range(2):
                po = psum.tile([P, 512], F32, tag="av", bufs=4, name=f"poa_{qb}_{n}")
                po_av[(qb, n)] = po
                for h in range(3):
                    nc.tensor.matmul(
                        po[:],
                        sumT[:, h, qb * P : (qb + 1) * P],
                        wfT[:, h, n * 512 : (n + 1) * 512],
                        start=(h == 0),
                        stop=False,
                    )

        def out_dma(eng, qb, n, ot):
            eng.dma_start(
                out=out[qb * P : (qb + 1) * P, n * 512 : (n + 1) * 512],
                in_=ot[:],
            )

        engs = [nc.sync, nc.scalar]
        for i, ((qb, n), po) in enumerate(po_av.items()):
            for h in range(3, H):
                nc.tensor.matmul(
                    po[:],
                    sumT[:, h, qb * P : (qb + 1) * P],
                    wfT[:, h, n * 512 : (n + 1) * 512],
                    start=False,
                    stop=(h == H - 1),
                )
            ot = outp.tile([P, 512], F32, tag="out", name=f"ota_{qb}_{n}")
            nc.any.tensor_copy(out=ot[:], in_=po[:])
            out_dma(engs[i % 2], qb, n, ot)
        for n in range(2):
            po = mm_tile(f"po_b_{n}")
            for half in range(2):
                qb = 2 + half
                for h in range(H):
                    nc.tensor.matmul(
                        po[:, half, :],
                        sumT[:, h, qb * P : (qb + 1) * P],
                        wfT[:, h, n * 512 : (n + 1) * 512],
                        start=(h == 0),
                        stop=(h == H - 1),
                    )
            for half in range(2):
                qb = 2 + half
                ot = outp.tile([P, 512], F32, tag="out", name=f"otb_{qb}_{n}")
                nc.any.tensor_copy(out=ot[:], in_=po[:, half, :])
                out_dma(engs[(qb + n) % 2], qb, n, ot)

    nc.compile()
    return nc


_NC_CACHE = None


def _get_nc():
    global _NC_CACHE
    if _NC_CACHE is None:
        _NC_CACHE = build_attention_kernel()
    return _NC_CACHE


def make_in_maps(inputs):
    inputs = {k: np.asarray(v) for k, v in inputs.items()}
    in_maps = []
    for c in range(NCORES):
        b, s = c // 4, c % 4
        in_maps.append(
            {
                "xq": np.ascontiguousarray(inputs["inputs_q"][b]),
                "xk": np.ascontiguousarray(
                    inputs["inputs_k"][b, s * KSH : (s + 1) * KSH]
                ),
                "xv": np.ascontiguousarray(
                    inputs["inputs_v"][b, s * KSH : (s + 1) * KSH]
                ),
                "msk": np.ascontiguousarray(
                    inputs["attention_mask"][b, :, s * KSH : (s + 1) * KSH]
                ).view(np.uint8),
                "wq": np.ascontiguousarray(inputs["Wq"]),
                "wk": np.ascontiguousarray(inputs["Wk"]),
                "wv": np.ascontiguousarray(inputs["Wv"]),
                "wf": np.ascontiguousarray(inputs["Wf"]),
            }
        )
    return in_maps


def gather_out(results):
    out = np.zeros((B, QL, D), np.float32)
    for c in range(NCORES):
        out[c // 4] += results[c]["out"]
    return out


def kernel(**inputs) -> np.ndarray:
    ensure_ntff_hook()  # defensive: BASS_TRACE=1 in env would need the shim
    from concourse.bass_utils import run_bass_kernel_spmd

    nc = _get_nc()
    in_maps = make_in_maps(inputs)
    res = run_bass_kernel_spmd(nc, in_maps, list(range(NCORES)))
    return gather_out(res.results)



# revision 8
# speedup vs baseline: 1.0075x; 1.0075x over previous
"""Multi-head attention kernel for Trainium2, SPMD over 8 NeuronCores.

Sharding: data-parallel over batch (2 groups of 4 cores) x sequence-parallel
over the key/value length within each group (4 slices of 2048). Each core
computes, for its (batch, k-slice): Q/K/V projections (all heads), masked
softmax numerators/denominators over its k-slice, the attention-weighted
values, and a partial final projection. Denominators are AllReduce'd within
each 4-core group on device (split 7 heads + 1 so the second hides under the
final projection); the 4 partial projected outputs per batch are summed on
the host.

All activations and weights are pre-transposed and pre-cast to bf16 on the
host, so the device does zero PE transposes and reads half the HBM bytes:
every matmul operand already has its contraction dim on partitions. The AV
matmul runs with V stationary and probs moving (free dim 512), producing
num^T [hd, q] directly in the layout the final projection consumes. The
softmax denominator is a DVE accumulation of masked probs folded by a
single M=1 ones-matmul per head; no max-subtraction is needed (scores are
O(1)) and masking is a multiplicative bf16 0/1 mask applied after exp.
"""

import sys

if "/opt/trn_rl_repo" not in sys.path:
    sys.path.insert(0, "/opt/trn_rl_repo")

from contextlib import ExitStack

import numpy as np

import concourse.bass as bass
import concourse.mybir as mybir
import concourse.tile as tile
from concourse import bacc

B, QL, KL, D, H = 2, 512, 8192, 1024, 8
HD = D // H  # 128
NCORES = 8
GROUPS = [[0, 1, 2, 3], [4, 5, 6, 7]]
KSH = KL // 4  # 2048 k rows per core
SCALE = 1.0 / float(np.sqrt(HD))

F32 = mybir.dt.float32
BF16 = mybir.dt.bfloat16
P = 128
KC = KSH // P  # 16 k chunks of 128
NKB = KSH // 512  # 4 k-row blocks of 512 for the projections
QB = QL // P  # 4 q blocks


def ensure_ntff_hook():
    """Provide antenv.axon_hooks (missing in this image) so trace=True works.

    Mirrors trn_agent_boot._ntff_profile_via_ctypes against the local
    libaxon_pjrt.so. No-op if the real module exists or the .so is absent.
    """
    try:
        import antenv.axon_hooks  # noqa: F401

        return
    except ImportError:
        pass
    import contextlib
    import ctypes
    import types

    mod = types.ModuleType("antenv.axon_hooks")
    holder = [None]
    mod.set_axon_ntff_profile_hook = lambda h: holder.__setitem__(0, h)
    mod.get_axon_ntff_profile_hook = lambda: holder[0]
    try:
        lib = ctypes.CDLL("/opt/axon/libaxon_pjrt.so")
        if hasattr(lib, "axon_start_nrt_profile"):
            lib.axon_start_nrt_profile.argtypes = [
                ctypes.POINTER(ctypes.c_int64),
                ctypes.c_size_t,
            ]
            lib.axon_start_nrt_profile.restype = ctypes.c_int64
            lib.axon_stop_nrt_profile.argtypes = [ctypes.c_char_p]
            lib.axon_stop_nrt_profile.restype = ctypes.c_int64

            @contextlib.contextmanager
            def _hook(output_dir, device_ids):
                import jax

                jax.devices()
                if device_ids:
                    ids = (ctypes.c_int64 * len(device_ids))(*device_ids)
                    rc = lib.axon_start_nrt_profile(ids, len(device_ids))
                else:
                    rc = lib.axon_start_nrt_profile(None, 0)
                if rc != 0:
                    raise RuntimeError(f"axon_start_nrt_profile rc={rc}")
                try:
                    yield
                finally:
                    n = lib.axon_stop_nrt_profile(str(output_dir).encode())
                    print(f"ntff profile: {n} file(s) -> {output_dir}")

            holder[0] = _hook
    except OSError:
        pass
    sys.modules["antenv.axon_hooks"] = mod
    try:
        import antenv

        antenv.axon_hooks = mod
    except ImportError:
        pass


def build_attention_kernel():
    nc = bacc.Bacc(
        "TRN2", target_bir_lowering=False, debug=False, num_devices=NCORES
    )

    # All inputs pre-transposed ([contraction, free]) and pre-cast on host.
    xqT = nc.declare_dram_parameter("xqT", [D, QL], BF16, isOutput=False)
    xkT = nc.declare_dram_parameter("xkT", [D, KSH], BF16, isOutput=False)
    xvT = nc.declare_dram_parameter("xvT", [D, KSH], BF16, isOutput=False)
    mskT = nc.declare_dram_parameter("mskT", [KSH, QL], BF16, isOutput=False)
    wqT = nc.declare_dram_parameter("wqT", [D, D], BF16, isOutput=False)
    wkT = nc.declare_dram_parameter("wkT", [D, D], BF16, isOutput=False)
    wvT = nc.declare_dram_parameter("wvT", [D, D], BF16, isOutput=False)
    wfT = nc.declare_dram_parameter("wfT", [D, D], BF16, isOutput=False)
    out = nc.declare_dram_parameter("out", [QL, D], F32, isOutput=True)

    with tile.TileContext(nc) as tc, ExitStack() as ctx:
        consts = ctx.enter_context(tc.tile_pool(name="consts", bufs=1))
        ones_col = consts.tile([P, 1], BF16)
        nc.vector.memset(ones_col, 1.0)
        ones_row = consts.tile([1, P], BF16)
        nc.vector.memset(ones_row, 1.0)

        persist = ctx.enter_context(tc.tile_pool(name="persist", bufs=1))
        qT = persist.tile([P, H, QL], BF16)  # [hd, h, q]
        kT = persist.tile([P, H, KSH], BF16)  # [hd, h, krow]
        v_sb = persist.tile([P, KC, D], BF16)  # [krow, kc, (h hd)]
        mk = persist.tile([P, KC, QL], BF16)  # [krow, kc, q]
        num_sb = persist.tile([P, H, QL], BF16)  # [hd, h, q] unnormalized
        rden = persist.tile([P, H, QL], BF16)  # [*, h, q] 1/den bcast
        sumT = persist.tile([P, H, QL], BF16)  # [hd, h, q] normalized
        den_rows = persist.tile([1, H * QL], BF16)  # AR result staging

        wts = ctx.enter_context(tc.tile_pool(name="wts", bufs=2))
        xs = ctx.enter_context(tc.tile_pool(name="xs", bufs=3))
        probs_pool = ctx.enter_context(tc.tile_pool(name="probs", bufs=3))
        acc_pool = ctx.enter_context(tc.tile_pool(name="accp", bufs=2))
        dens_pool = ctx.enter_context(tc.tile_pool(name="dens", bufs=2))
        outp = ctx.enter_context(tc.tile_pool(name="outp", bufs=2))
        dram = ctx.enter_context(tc.tile_pool(name="dram", bufs=1, space="DRAM"))
        psum = ctx.enter_context(tc.tile_pool(name="psum", bufs=1, space="PSUM"))

        def mm_tile(name):
            return psum.tile([P, 2, 512], F32, tag="mm", bufs=2, name=name)

        def aux_tile(name):
            return psum.tile([P, 512], F32, tag="aux", bufs=2, name=name)

        # ---- input DMA streams -------------------------------------------
        # gpsimd queue: the projection-critical path, in consumption order.
        def load_w(w_dram, name):
            t = wts.tile([P, H, D], BF16, tag="w", name=name)
            for hf in range(2):
                nc.gpsimd.dma_start(
                    out=t[:, hf * 4 : (hf + 1) * 4, :],
                    in_=w_dram[hf * 512 : (hf + 1) * 512, :].rearrange(
                        "(c p) d -> p c d", p=P
                    ),
                )
            return t

        wq_sb = load_w(wqT, "wq")
        xq_sb = xs.tile([P, H, QL], BF16, tag="x", name="xq")
        nc.gpsimd.dma_start(out=xq_sb, in_=xqT.rearrange("(c p) q -> p c q", p=P))
        wk_sb = load_w(wkT, "wk")
        xk_blk = []
        for kb in range(NKB):
            t = xs.tile([P, H, 512], BF16, tag="x", name=f"xk{kb}")
            nc.gpsimd.dma_start(
                out=t,
                in_=xkT[:, kb * 512 : (kb + 1) * 512].rearrange(
                    "(c p) k -> p c k", p=P
                ),
            )
            xk_blk.append(t)
        wv_sb = load_w(wvT, "wv")
        xv_blk = []
        for kb in range(NKB):
            t = xs.tile([P, H, 512], BF16, tag="x", name=f"xv{kb}")
            nc.gpsimd.dma_start(
                out=t,
                in_=xvT[:, kb * 512 : (kb + 1) * 512].rearrange(
                    "(c p) k -> p c k", p=P
                ),
            )
            xv_blk.append(t)

        # sync queue: mask (needed from the first attention head) + Wf (end).
        for hf in range(2):
            nc.sync.dma_start(
                out=mk[:, hf * 8 : (hf + 1) * 8, :],
                in_=mskT[hf * 1024 : (hf + 1) * 1024, :].rearrange(
                    "(c p) q -> p c q", p=P
                ),
            )
        wf_sb = wts.tile([P, H, D], BF16, tag="w", name="wf")
        for hf in range(2):
            nc.sync.dma_start(
                out=wf_sb[:, hf * 4 : (hf + 1) * 4, :],
                in_=wfT[hf * 512 : (hf + 1) * 512, :].rearrange(
                    "(c p) d -> p c d", p=P
                ),
            )

        # ---- Q projection: qT[hd, h, q] ----------------------------------
        for m2 in range(H // 2):
            ps = mm_tile(f"pq{m2}")
            for half in range(2):
                m = m2 * 2 + half
                for cc in range(H):
                    nc.tensor.matmul(
                        ps[:, half, :],
                        wq_sb[:, cc, m * P : (m + 1) * P],
                        xq_sb[:, cc, :],
                        start=(cc == 0),
                        stop=(cc == H - 1),
                    )
            nc.any.tensor_copy(out=qT[:, m2 * 2 : m2 * 2 + 2, :], in_=ps[:])

        # ---- K projection: kT[hd, h, krow], streamed per 512-row block ---
        for kb in range(NKB):
            for m2 in range(H // 2):
                ps = mm_tile(f"pk{kb}_{m2}")
                for half in range(2):
                    m = m2 * 2 + half
                    for cc in range(H):
                        nc.tensor.matmul(
                            ps[:, half, :],
                            wk_sb[:, cc, m * P : (m + 1) * P],
                            xk_blk[kb][:, cc, :],
                            start=(cc == 0),
                            stop=(cc == H - 1),
                        )
                nc.any.tensor_copy(
                    out=kT[:, m2 * 2 : m2 * 2 + 2, kb * 512 : (kb + 1) * 512],
                    in_=ps[:],
                )

        # ---- V projection: v_sb[krow, kc, dout] --------------------------
        for kt in range(KC):
            kb, j = kt // 4, kt % 4
            ps = mm_tile(f"pv{kt}")
            for n in range(2):
                for cc in range(H):
                    nc.tensor.matmul(
                        ps[:, n, :],
                        xv_blk[kb][:, cc, j * P : (j + 1) * P],
                        wv_sb[:, cc, n * 512 : (n + 1) * 512],
                        start=(cc == 0),
                        stop=(cc == H - 1),
                    )
            nc.any.tensor_copy(
                out=v_sb[:, kt, :], in_=ps[:].rearrange("p a b -> p (a b)")
            )

        # ---- attention per head ------------------------------------------
        den_in_a = dram.tile([H - 1, 512], F32, name="den_in_a")
        den_out_a = dram.tile([H - 1, 512], F32, name="den_out_a")
        den_in_b = dram.tile([1, 512], F32, name="den_in_b")
        den_out_b = dram.tile([1, 512], F32, name="den_out_b")

        for h in range(H):
            av = psum.tile([P, 512], F32, tag="av", bufs=2, name=f"av{h}")
            acc = acc_pool.tile([P, 2, 512], BF16, tag="acc", name=f"acc{h}")
            for kc2 in range(KC // 2):
                ps = mm_tile(f"ps{h}_{kc2}")
                for half in range(2):
                    kc = kc2 * 2 + half
                    nc.tensor.matmul(
                        ps[:, half, :],
                        kT[:, h, kc * P : (kc + 1) * P],
                        qT[:, h, :],
                        start=True,
                        stop=True,
                    )
                pr = probs_pool.tile(
                    [P, 2, 512], BF16, tag="pr", name=f"pr{h}_{kc2}"
                )
                nc.scalar.activation(
                    pr[:], ps[:], mybir.ActivationFunctionType.Exp, scale=SCALE
                )
                if kc2 == 0:
                    # first chunk's masked probs double as the accumulator init
                    nc.vector.tensor_mul(
                        acc[:], pr[:], mk[:, kc2 * 2 : kc2 * 2 + 2, :]
                    )
                    pr = acc
                else:
                    nc.vector.tensor_mul(
                        pr[:], pr[:], mk[:, kc2 * 2 : kc2 * 2 + 2, :]
                    )
                    with nc.allow_low_precision(
                        reason="bf16 softmax-denominator accumulation; "
                        "only used for normalization, ~0.4% rel error"
                    ):
                        nc.vector.tensor_add(acc[:], acc[:], pr[:])
                for half in range(2):
                    kc = kc2 * 2 + half
                    nc.tensor.matmul(
                        av[:],
                        v_sb[:, kc, h * HD : (h + 1) * HD],
                        pr[:, half, :],
                        start=(kc == 0),
                        stop=(kc == KC - 1),
                    )
            # fold the accumulator over partitions -> den row [1, 512]
            dn = aux_tile(f"dn{h}")
            for half in range(2):
                nc.tensor.matmul(
                    dn[0:1, :],
                    ones_col,
                    acc[:, half, :],
                    start=(half == 0),
                    stop=(half == 1),
                )
            ds = dens_pool.tile([1, 512], F32, tag="ds", name=f"ds{h}")
            nc.any.tensor_copy(out=ds[:], in_=dn[0:1, :])
            if h < H - 1:
                nc.sync.dma_start(out=den_in_a[h : h + 1, :], in_=ds[:])
            else:
                nc.sync.dma_start(out=den_in_b[0:1, :], in_=ds[:])
            nc.any.tensor_copy(out=num_sb[:, h, :], in_=av[:])
            if h == H - 2:
                nc.gpsimd.collective_compute(
                    "AllReduce",
                    mybir.AluOpType.add,
                    replica_groups=GROUPS,
                    ins=[den_in_a.opt()],
                    outs=[den_out_a.opt()],
                )
            if h == H - 1:
                nc.gpsimd.collective_compute(
                    "AllReduce",
                    mybir.AluOpType.add,
                    replica_groups=GROUPS,
                    ins=[den_in_b.opt()],
                    outs=[den_out_b.opt()],
                )

        # den rows back to SBUF (cast to bf16 in the DMA)
        nc.gpsimd.dma_start(
            out=den_rows[:, 0 : (H - 1) * 512],
            in_=den_out_a.rearrange("(o a) b -> o (a b)", o=1),
        )
        nc.gpsimd.dma_start(
            out=den_rows[:, (H - 1) * 512 :],
            in_=den_out_b.rearrange("(o a) b -> o (a b)", o=1),
        )

        # rden[*, h, q] = 1/max(den, eps), broadcast across partitions
        def norm_head(h):
            bc = aux_tile(f"bc{h}")
            nc.tensor.matmul(
                bc[:],
                ones_row,
                den_rows[:, h * 512 : (h + 1) * 512],
                start=True,
                stop=True,
            )
            nc.vector.tensor_scalar_max(bc[:], bc[:], 1e-30)
            with nc.allow_low_precision(
                reason="bf16 reciprocal of softmax denominator; ~0.4% rel"
            ):
                nc.vector.reciprocal(rden[:, h, :], bc[:])
            nc.vector.tensor_mul(sumT[:, h, :], num_sb[:, h, :], rden[:, h, :])

        for h in range(H - 1):
            norm_head(h)

        # ---- final projection + output -----------------------------------
        # heads 0..6 accumulate while the second AllReduce is in flight
        engs = [nc.sync, nc.scalar]
        po_tiles = []
        for qt in range(2):
            po = mm_tile(f"po{qt}")
            po_tiles.append(po)
            for h in range(H - 1):
                for n in range(2):
                    nc.tensor.matmul(
                        po[:, n, :],
                        sumT[:, h, qt * P : (qt + 1) * P],
                        wf_sb[:, h, n * 512 : (n + 1) * 512],
                        start=(h == 0),
                        stop=False,
                    )
        norm_head(H - 1)
        for qt in range(2):
            po = po_tiles[qt]
            for n in range(2):
                nc.tensor.matmul(
                    po[:, n, :],
                    sumT[:, H - 1, qt * P : (qt + 1) * P],
                    wf_sb[:, H - 1, n * 512 : (n + 1) * 512],
                    start=False,
                    stop=True,
                )
            ot = outp.tile([P, 2, 512], F32, tag="out", name=f"ot{qt}")
            nc.any.tensor_copy(out=ot[:], in_=po[:])
            engs[qt % 2].dma_start(
                out=out[qt * P : (qt + 1) * P, :],
                in_=ot[:].rearrange("p a b -> p (a b)"),
            )
        for qt in range(2, QB):
            po = mm_tile(f"po{qt}")
            for h in range(H):
                for n in range(2):
                    nc.tensor.matmul(
                        po[:, n, :],
                        sumT[:, h, qt * P : (qt + 1) * P],
                        wf_sb[:, h, n * 512 : (n + 1) * 512],
                        start=(h == 0),
                        stop=(h == H - 1),
                    )
            ot = outp.tile([P, 2, 512], F32, tag="out", name=f"ot{qt}")
            nc.any.tensor_copy(out=ot[:], in_=po[:])
            engs[qt % 2].dma_start(
                out=out[qt * P : (qt + 1) * P, :],
                in_=ot[:].rearrange("p a b -> p (a b)"),
            )

    nc.compile()
    return nc


_NC_CACHE = None


def _get_nc():
    global _NC_CACHE
    if _NC_CACHE is None:
        _NC_CACHE = build_attention_kernel()
    return _NC_CACHE


def make_in_maps(inputs):
    import ml_dtypes

    bf16 = ml_dtypes.bfloat16
    inputs = {k: np.asarray(v) for k, v in inputs.items()}

    def t(a):
        return np.ascontiguousarray(np.asarray(a, np.float32).T.astype(bf16))

    wq_t = t(inputs["Wq"])  # [din, dout]
    wk_t = t(inputs["Wk"])
    wv_t = t(inputs["Wv"])
    wf_t = t(inputs["Wf"])
    in_maps = []
    for c in range(NCORES):
        b, s = c // 4, c % 4
        sl = slice(s * KSH, (s + 1) * KSH)
        in_maps.append(
            {
                "xqT": t(inputs["inputs_q"][b]),
                "xkT": t(inputs["inputs_k"][b, sl]),
                "xvT": t(inputs["inputs_v"][b, sl]),
                "mskT": np.ascontiguousarray(
                    inputs["attention_mask"][b, :, sl].T
                ).astype(bf16),
                "wqT": wq_t,
                "wkT": wk_t,
                "wvT": wv_t,
                "wfT": wf_t,
            }
        )
    return in_maps


def gather_out(results):
    out = np.zeros((B, QL, D), np.float32)
    for c in range(NCORES):
        out[c // 4] += results[c]["out"]
    return out


def kernel(**inputs) -> np.ndarray:
    ensure_ntff_hook()  # defensive: BASS_TRACE=1 in env would need the shim
    from concourse.bass_utils import run_bass_kernel_spmd

    nc = _get_nc()
    in_maps = make_in_maps(inputs)
    res = run_bass_kernel_spmd(nc, in_maps, list(range(NCORES)))
    return gather_out(res.results)
